# revision 2
# baseline (speedup 1.0000x reference)
"""GQA attention kernel for 8 TRN2 NeuronCores.

Problem: B=2, N=2048, DIM=1024, 16 q-heads / 4 kv-heads, head dim 64.
Sharding: core c handles batch c//4 and kv-head group c%4 (4 q-heads that
share one kv head).  Wq/Wk/Wv column-sharded, Wo row-sharded; the Wo row
reduction (4 cores per batch) and the bias add happen on the host.

Per-core algorithm (everything transposed so no on-chip transposes needed):
  QT = Wq_shard.T @ x.T          [256, 2048]   (4 heads stacked as 2x128)
  KT = Wk_dup.T  @ x.T           [128, 2048]   (kv head duplicated twice)
  V  = x @ Wv_shard              [2048, 64] -> bf16, + ones column (row sums)
  per head:  S^T tile = K Q_h^T ; E = exp(S^T/8) (bf16) ; P = E * keepT
             O_aug^T += V_aug^T @ P  (PSUM accum over key chunks)
             row 64 of O_aug^T = softmax denominators s
             r = exp(-ln(s)), broadcast over dh via a K=1 ones-matmul
             OTn = O^T * r
  out_partial = concat_heads(OTn).T @ Wo_shard   (K=128 per head pair)
"""

import sys

for _p in ("/opt/trn_rl_repo",):
    if _p not in sys.path:
        sys.path.insert(0, _p)

import numpy as np
import ml_dtypes

import concourse.bass as bass  # noqa: F401  (registers AP machinery)
import concourse.tile as tile
from concourse import bacc, mybir
from concourse.bass_utils import run_bass_kernel_spmd

F32 = mybir.dt.float32
F32R = mybir.dt.float32r
BF16 = mybir.dt.bfloat16
EXP = mybir.ActivationFunctionType.Exp
LN = mybir.ActivationFunctionType.Ln

B, NTOK, DIM = 2, 2048, 1024
H, KVH, DH = 16, 4, 64
P = 128
TQ = 1024  # q-block width for the attention inner loop
SCALE = DH ** -0.5

N_CORES = 8

import concourse.bacc as _bacc_mod
import concourse.hw_specs as _hw_specs

_ORIG_GAT = _hw_specs.get_activation_tables


def _gat_combined(arch):
    tables = _ORIG_GAT(arch)
    if any(n == "natural_log_exp_and_others" for n in tables):
        for name, funcs in tables.items():
            if name != "natural_log_exp_and_others":
                funcs.discard(EXP)
                funcs.discard(LN)
    return tables


_bacc_mod.get_activation_tables = _gat_combined


def _build_kernel():
    nc = bacc.Bacc("TRN2", target_bir_lowering=False, debug=False,
                   num_devices=N_CORES)

    xT_d = nc.dram_tensor("xT", [DIM, NTOK], BF16, kind="ExternalInput")
    kT_d = nc.dram_tensor("keepT", [NTOK, NTOK], BF16, kind="ExternalInput")
    wq_d = nc.dram_tensor("wq", [DIM, 256], BF16, kind="ExternalInput")
    wk_d = nc.dram_tensor("wk2", [DIM, 128], BF16, kind="ExternalInput")
    wv_d = nc.dram_tensor("wv", [DIM, DH], BF16, kind="ExternalInput")
    wo_d = nc.dram_tensor("wo", [256, DIM], F32R, kind="ExternalInput")
    out_d = nc.dram_tensor("out", [NTOK, DIM], F32, kind="ExternalOutput")

    with tile.TileContext(nc) as tc:
        with tc.tile_pool(name="persist", bufs=1) as pp, \
             tc.tile_pool(name="work", bufs=3) as wp, \
             tc.tile_pool(name="otnp", bufs=2) as op_, \
             tc.tile_pool(name="psA", bufs=1, space="PSUM") as psA:

            # ---- resident tensors -------------------------------------
            # small weights first so projection matmuls can start as soon
            # as the first xT chunks land; xT spread over two DMA queues.
            wq = pp.tile([P, 8, 256], BF16, tag="wq")
            nc.sync.dma_start(wq[:], wq_d.ap().rearrange("(o p) m -> p o m", p=P))
            wk = pp.tile([P, 8, P], BF16, tag="wk")
            nc.gpsimd.dma_start(wk[:], wk_d.ap().rearrange("(o p) m -> p o m", p=P))
            wv = pp.tile([P, 8, DH], BF16, tag="wv")
            nc.gpsimd.dma_start(wv[:], wv_d.ap().rearrange("(o p) m -> p o m", p=P))
            xT = pp.tile([P, 8, NTOK], BF16, tag="xT")
            for o in range(8):
                eng = nc.sync if o % 2 == 0 else nc.gpsimd
                eng.dma_start(xT[:, o, :], xT_d[o * P:(o + 1) * P, :])
            # wo2[p, mm, :]: rows h01*64+d of pair mm  (h01 = p // 64)
            wo2 = pp.tile([P, 2, DIM], F32R, tag="wo2")
            for mm in range(2):
                for h01 in range(2):
                    hh = 2 * mm + h01
                    nc.sync.dma_start(wo2[h01 * 64:(h01 + 1) * 64, mm, :],
                                      wo_d[hh * 64:(hh + 1) * 64, :])

            # ---- projections ------------------------------------------
            QT = pp.tile([P, 2, NTOK], F32R, tag="QT")
            for m in range(2):
                for n in range(4):
                    ps = psA.tile([P, 512], F32, tag=f"o{n % 2}")
                    for d in range(8):
                        nc.tensor.matmul(ps[:],
                                         lhsT=(wq[:, d, m * P:(m + 1) * P]),
                                         rhs=(xT[:, d, n * 512:(n + 1) * 512]),
                                         start=(d == 0), stop=(d == 7))
                    nc.vector.tensor_copy(out=QT[:, m, n * 512:(n + 1) * 512],
                                          in_=ps[:])
            KT = pp.tile([P, NTOK], F32R, tag="KT")
            for n in range(4):
                ps = psA.tile([P, 512], F32, tag=f"o{n % 2}")
                for d in range(8):
                    nc.tensor.matmul(ps[:], lhsT=(wk[:, d, :]),
                                     rhs=(xT[:, d, n * 512:(n + 1) * 512]),
                                     start=(d == 0), stop=(d == 7))
                nc.vector.tensor_copy(out=KT[:, n * 512:(n + 1) * 512], in_=ps[:])
            Vb = pp.tile([P, 16, DH + 1], BF16, tag="Vb")
            for t in range(16):
                ps = psA.tile([P, DH], F32, tag=f"o{t % 2}")
                for d in range(8):
                    nc.tensor.matmul(ps[:],
                                     lhsT=(xT[:, d, t * P:(t + 1) * P]),
                                     rhs=(wv[:, d, :]),
                                     start=(d == 0), stop=(d == 7))
                nc.vector.tensor_copy(out=Vb[:, t, 0:DH], in_=ps[:])
                nc.vector.memset(Vb[:, t, DH:DH + 1], 1.0)

            # ---- attention --------------------------------------------
            # qc outer / head-pair inner.  Normalization and the output
            # projection are emitted at block boundaries (engines execute
            # their streams in order, so mid-block emission of ops that
            # wait on this block's PSUM slots would deadlock).
            otn_tiles = [op_.tile([P, NTOK], F32R, tag="otn", name=f"otn{m}")
                         for m in range(2)]

            ones = pp.tile([1, 64], F32, tag="ones")
            nc.vector.memset(ones[:], 1.0)

            def emit_norm(u, otn, h, qc):
                lns = wp.tile([1, TQ], F32, tag="rrow")
                nc.scalar.activation(lns[:], u[DH:DH + 1, :], LN)
                pb = psA.tile([64, TQ], F32, tag="s", bufs=2, name="pb")
                for qh in range(2):
                    nc.tensor.matmul(
                        pb[:, qh * 512:(qh + 1) * 512],
                        lhsT=ones[:],
                        rhs=lns[:, qh * 512:(qh + 1) * 512],
                        start=True, stop=True)
                rb = wp.tile([64, TQ], F32, tag="rb")
                nc.scalar.activation(rb[:], pb[:], EXP, scale=-1.0)
                nc.vector.tensor_mul(
                    out=otn[h * 64:(h + 1) * 64, qc * TQ:(qc + 1) * TQ],
                    in0=u[0:DH, :], in1=rb[:])

            def emit_proj(t, tail=False):
                for n2 in range(2):
                    pf = psA.tile([P, 512], F32, tag=f"o{n2}",
                                  name=f"pf{t}_{n2}")
                    for mm in range(2):
                        nc.tensor.matmul(
                            pf[:],
                            lhsT=(otn_tiles[mm][:, t * P:(t + 1) * P]),
                            rhs=(wo2[:, mm, n2 * 512:(n2 + 1) * 512]),
                            start=(mm == 0), stop=(mm == 1))
                    of = wp.tile([P, 512], F32, tag="of")
                    if tail and n2 == 1:
                        nc.scalar.copy(out=of[:], in_=pf[:])
                    else:
                        nc.vector.tensor_copy(out=of[:], in_=pf[:])
                    nc.sync.dma_start(
                        out_d[t * P:(t + 1) * P, n2 * 512:(n2 + 1) * 512],
                        of[:])

            pending_norm = []  # (u, otn, h, qc) staged but not yet normalized
            for qc in range(2):     # 1024-wide q block
                for m in range(2):  # head pair (heads 2m, 2m+1 of this core)
                    otn = otn_tiles[m]
                    po = [psA.tile([DH + 1, TQ], F32, tag=f"o{h}",
                                   name=f"po{h}")
                          for h in range(2)]
                    for kc in range(16):  # 128-wide key chunk
                        if kc == 4:
                            # previous block's normalization, interleaved here
                            # so its ACT/DVE/PE work fills pipeline slack
                            # instead of stalling the block boundary.  Only
                            # the fast-churning "s" PSUM tag is touched, so
                            # there is no slot deadlock against live po tiles.
                            for args in pending_norm:
                                emit_norm(*args)
                            pending_norm.clear()
                        kt = wp.tile([P, TQ], BF16, tag="kt", bufs=4)
                        nc.sync.dma_start(
                            kt[:], kT_d[kc * P:(kc + 1) * P,
                                        qc * TQ:(qc + 1) * TQ])
                        for h in range(2):
                            ss = psA.tile([P, TQ], F32, tag="s", bufs=2)
                            for qh in range(2):
                                nc.tensor.matmul(
                                    ss[:, qh * 512:(qh + 1) * 512],
                                    lhsT=(KT[h * 64:(h + 1) * 64,
                                               kc * P:(kc + 1) * P]),
                                    rhs=(QT[h * 64:(h + 1) * 64, m,
                                              qc * TQ + qh * 512:
                                              qc * TQ + (qh + 1) * 512]),
                                    start=True, stop=True)
                            ee = wp.tile([P, TQ], BF16, tag="ee")
                            nc.scalar.activation(ee[:], ss[:], EXP, scale=SCALE)
                            pt = wp.tile([P, TQ], BF16, tag="pt")
                            nc.vector.tensor_mul(out=pt[:], in0=ee[:], in1=kt[:])
                            for qh in range(2):
                                nc.tensor.matmul(
                                    po[h][:, qh * 512:(qh + 1) * 512],
                                    lhsT=Vb[:, kc, :],
                                    rhs=pt[:, qh * 512:(qh + 1) * 512],
                                    start=(kc == 0), stop=(kc == 15))
                    # stage O_aug^T out of PSUM promptly (frees accumulator
                    # slots for the next block); normalization is deferred
                    # into the next block's key loop.
                    for h in range(2):
                        u = wp.tile([DH + 1, TQ], F32, tag="u", bufs=4)
                        nc.vector.tensor_copy(out=u[:], in_=po[h][:])
                        pending_norm.append((u, otn, h, qc))
                    # spread the first q-block's output projection over both
                    # second-qc boundaries to shrink the serial tail (their
                    # otn inputs were normalized during earlier key loops).
                    if qc == 1:
                        for t in range(m * 4, m * 4 + 4):
                            emit_proj(t)
            for args in pending_norm:
                emit_norm(*args)
            pending_norm.clear()
            for t in range(8, 16):
                emit_proj(t, tail=True)

    nc.compile()
    return nc


_NC_CACHE = None
_LAST_PARTS = None


def _get_nc():
    global _NC_CACHE
    if _NC_CACHE is None:
        _NC_CACHE = _build_kernel()
    return _NC_CACHE


def _prep_in_maps(x, mask, Wq, Wk, Wv, Wo, bo):
    x = np.asarray(x, dtype=np.float32)
    mask = np.asarray(mask)
    Wq = np.asarray(Wq, dtype=np.float32)
    Wk = np.asarray(Wk, dtype=np.float32)
    Wv = np.asarray(Wv, dtype=np.float32)
    Wo = np.asarray(Wo, dtype=np.float32)

    keepT = np.ascontiguousarray((~mask.astype(bool)).T).astype(ml_dtypes.bfloat16)
    in_maps = []
    for c in range(N_CORES):
        b, j = c // 4, c % 4
        in_maps.append({
            "xT": np.ascontiguousarray(x[b].T).astype(ml_dtypes.bfloat16),
            "keepT": keepT,
            "wq": np.ascontiguousarray(Wq[:, j * 256:(j + 1) * 256]).astype(ml_dtypes.bfloat16),
            "wk2": np.ascontiguousarray(
                np.concatenate([Wk[:, j * DH:(j + 1) * DH]] * 2,
                               axis=1)).astype(ml_dtypes.bfloat16),
            "wv": np.ascontiguousarray(Wv[:, j * DH:(j + 1) * DH]).astype(ml_dtypes.bfloat16),
            "wo": np.ascontiguousarray(Wo[j * 256:(j + 1) * 256, :]),
        })
    return in_maps


def _assemble(parts, bo):
    parts = [np.asarray(parts[c]).astype(np.float32) for c in range(N_CORES)]
    out = np.stack([parts[0] + parts[1] + parts[2] + parts[3],
                    parts[4] + parts[5] + parts[6] + parts[7]])
    out = out + np.asarray(bo, dtype=np.float32)[None, None, :]
    return out.astype(np.float32)


def kernel(x, mask, Wq, Wk, Wv, Wo, bo, _run_kwargs=None):
    nc = _get_nc()
    in_maps = _prep_in_maps(x, mask, Wq, Wk, Wv, Wo, bo)
    res = run_bass_kernel_spmd(nc, in_maps, list(range(N_CORES)),
                               **(_run_kwargs or {}))
    parts = [res.results[c]["out"] for c in range(N_CORES)]
    global _LAST_PARTS
    _LAST_PARTS = [np.asarray(p, dtype=np.float32) for p in parts]
    if _run_kwargs:
        kernel.last_results = res
    return _assemble(parts, bo)

